# revision 11
# baseline (speedup 1.0000x reference)
"""CRF Viterbi decode kernel for Trainium2 (8 NeuronCores, data-parallel over batch).

Per core (128 sequences, batch on partitions):
  Phase A: DMA X slabs (front/back interleaved) -> PE transpose -> PE matmul with W
           -> emissions e[b, s, l] (ACT copies PSUM->SBUF).
  Scan:    fwd delta-chain and bwd c-chain fused per iter, all on DVE
           (DVE+GpSimd share SBUF ports, so offloading elementwise work to
           GpSimd is zero-sum; a single in-order DVE queue has no sync
           stalls).  Iter i: sc = [T^T|T] + [delta_i|c_{511-i}] bcast,
           fused reduce -> [mx_f(i+1)|mx_b], fused out
           [delta_{i+1}|c_{510-i}] = mx + [e_{i+1}|e_{510-i}].
           mx_f(t) is stored per-t (mxf_store for t<288, dead e-slot 512-t
           after); c_t is stored per-t.  No ACT involvement in the scan.
  Tail:    gamma_t = mx_f(t) + c_t (GpSimd, chunked); onehot =
           first-wins(gamma == rowmax(gamma)) via segmented scan on DVE; DMA out.

gamma identity: delta_t = mx_f(t) + e_t, c_t = beta_t + e_t
  => delta_t + beta_t = mx_f(t) + c_t  (mx_f(0) := 0).
"""

import numpy as np

B, S, D, L = 1024, 512, 128, 26
NCORES = 8
BC = B // NCORES  # 128 sequences per core

_BUILD_CACHE = {}


def _build(s_len):
    import concourse.bass as bass
    import concourse.bacc as bacc
    import concourse.tile as tile
    import concourse.mybir as mybir

    alu = mybir.AluOpType
    f32 = mybir.dt.float32
    i32 = mybir.dt.int32

    nc = bacc.Bacc("TRN2", target_bir_lowering=False, debug=False)
    Xh = nc.dram_tensor("X", (BC, s_len, D), f32, kind="ExternalInput")
    Wh = nc.dram_tensor("W", (D, L), f32, kind="ExternalInput")
    Th = nc.dram_tensor("T", (L, L), f32, kind="ExternalInput")
    Oh = nc.dram_tensor("OUT", (BC, s_len, L), f32, kind="ExternalOutput")

    SCHUNK = 8 if s_len % 16 == 0 else s_len   # X staging granularity
    NCH = s_len // SCHUNK
    CH = 32 if s_len % 64 == 0 else s_len      # tail chunk (timesteps)
    NF = s_len - 1                             # fwd steps
    DSLOT = s_len                              # ctile ping-pong base
    SPLIT_I = 287                              # first iter whose mx_f -> e_store
    MXN = SPLIT_I + 2                          # mxf slots (0=zeros, MXN-1=scratch)
    SCR2 = 251                                 # late-phase bwd-mx scratch e-slot
    GROWS = 23                                 # sc_b rows computed by GpSimd

    def ap_of(t, offset_elems, dims):
        a = t[:]
        return bass.AP(tensor=a.tensor, offset=a.offset + offset_elems,
                       ap=[list(a.ap[0])] + dims)

    with tile.TileContext(nc) as tc:
        with (
            tc.tile_pool(name="singles", bufs=1) as singles,
            tc.tile_pool(name="xstage", bufs=2) as xstage_p,
            tc.tile_pool(name="xt", bufs=3) as xt_p,
            tc.tile_pool(name="ps_t", bufs=2, space="PSUM") as ps_t,
            tc.tile_pool(name="ps_e", bufs=2, space="PSUM") as ps_e,
            tc.tile_pool(name="scf", bufs=2) as scf_p,
            tc.tile_pool(name="scb", bufs=2) as scb_p,
            tc.tile_pool(name="gam", bufs=2) as gam_p,
            tc.tile_pool(name="tail1", bufs=1) as tail1,
            tc.tile_pool(name="tail2", bufs=2) as tail2,
        ):
            # ---- storage ----
            e_store = singles.tile([BC, s_len * L], f32)       # emissions
            ctile = singles.tile([BC, (s_len + 2) * L], f32)   # c_t + delta ping-pong
            mxf = singles.tile([BC, MXN * L], f32)             # mx_f(t), t <= SPLIT_I
            w_sb = singles.tile([D, L], f32)
            nc.sync.dma_start(w_sb[:], Wh[:])

            # t_cat[p, 0, j, i] = T[i, j]; t_cat[p, 1, i, j] = T[i, j]
            t_ap = Th[:]
            t_cat = singles.tile([BC, 2, L, L], f32)
            nc.sync.dma_start(
                t_cat[:, 1, :, :].rearrange("p a b -> p (a b)"),
                bass.AP(tensor=t_ap.tensor, offset=t_ap.offset,
                        ap=[[0, BC], [1, L * L]]),
            )
            nc.vector.tensor_copy(
                ap_of(t_cat, 0, [[L, L], [1, L]]),
                ap_of(t_cat, L * L, [[1, L], [L, L]]),
            )

            # identity matrix for PE transpose: ident[p, q] = (p == q)
            idx_i = singles.tile([BC, D], i32)
            nc.gpsimd.iota(idx_i[:], pattern=[[1, D]], base=0, channel_multiplier=0)
            pid_i = singles.tile([BC, 1], i32)
            nc.gpsimd.iota(pid_i[:], pattern=[[0, 1]], base=0, channel_multiplier=1)
            idx_f = singles.tile([BC, D], f32)
            nc.vector.tensor_copy(idx_f[:], idx_i[:])
            pid_f = singles.tile([BC, 1], f32)
            nc.vector.tensor_copy(pid_f[:], pid_i[:])
            ident = singles.tile([BC, D], f32)
            nc.vector.tensor_scalar(
                out=ident[:], in0=idx_f[:], scalar1=pid_f[:], scalar2=None,
                op0=alu.is_equal,
            )

            e3 = e_store.rearrange("p (s l) -> p s l", l=L)
            c3 = ctile.rearrange("p (s l) -> p s l", l=L)
            m3 = mxf.rearrange("p (s l) -> p s l", l=L)

            # ---- Phase A: emissions, front/back interleaved chunk order ----
            order = []
            for c in range(NCH // 2):
                order += [c, NCH - 1 - c]
            if NCH % 2:
                order.append(NCH // 2)
            for cidx in order:
                c0 = cidx * SCHUNK
                xs = xstage_p.tile([BC, SCHUNK, D], f32)
                nc.sync.dma_start(xs[:], Xh[:, c0:c0 + SCHUNK, :])
                for si in range(SCHUNK):
                    s = c0 + si
                    xt_psum = ps_t.tile([D, BC], f32)
                    nc.tensor.transpose(xt_psum[:], xs[:, si, :], ident[:])
                    xt_sb = xt_p.tile([D, BC], f32)
                    nc.scalar.copy(xt_sb[:], xt_psum[:])
                    e_psum = ps_e.tile([BC, L], f32)
                    nc.tensor.matmul(e_psum[:], lhsT=xt_sb[:], rhs=w_sb[:],
                                     start=True, stop=True)
                    nc.scalar.copy(e3[:, s, :], e_psum[:])

            # ---- init (DVE; the ACT queue is full of phase-A copies) ----
            nc.vector.memset(m3[:, 0, :], 0.0)                 # mx_f(0) := 0
            nc.vector.tensor_copy(c3[:, DSLOT, :], e3[:, 0, :])          # delta_0
            nc.vector.tensor_copy(c3[:, s_len - 1, :], e3[:, s_len - 1, :])  # c_{S-1}

            # bwd-step j: sc_b(j) = T + c_{511-j} bcast, red -> beta_{510-j},
            # c_{510-j} = beta + e_{510-j}.  GpSimd computes rows [0, GROWS)
            # (it runs at the port-contended rate while DVE holds the SBUF
            # ports); DVE fills the rest inside the consuming iter.
            def emit_scb(j):
                scb = scb_p.tile([BC, L, L], f32, tag="scb")
                nc.gpsimd.tensor_tensor(
                    scb[:, 0:GROWS, :],
                    ap_of(t_cat, L * L, [[L, GROWS], [1, L]]),
                    ap_of(ctile, (s_len - 1 - j) * L, [[0, GROWS], [1, L]]),
                    op=alu.add,
                )
                return scb

            scb_live = {0: emit_scb(0)}

            # ---- scan: iter i runs fwd-step i and bwd-step i-1 ----
            for i in range(NF + 1):
                fwd = i < NF
                j = i - 1
                ft = i + 1
                bt = s_len - 2 - j   # bwd step produced this iter
                din = DSLOT + (i % 2)
                dout = DSLOT + ((i + 1) % 2)
                if i < SPLIT_I:
                    mstore, mslot, sslot = mxf, ft, MXN - 1
                else:
                    mstore, mslot, sslot = e_store, s_len - i, SCR2

                if fwd:
                    # sc_f[a,b] = T[b,a] + delta_i[b]
                    scf = scf_p.tile([BC, L, L], f32, tag="scf")
                    nc.vector.tensor_tensor(
                        scf[:], t_cat[:, 0, :, :],
                        ap_of(ctile, din * L, [[0, L], [1, L]]),
                        op=alu.add,
                    )
                if j >= 0:
                    scb = scb_live.pop(j)
                    nc.vector.tensor_tensor(
                        scb[:, GROWS:L, :],
                        ap_of(t_cat, (L + GROWS) * L, [[L, L - GROWS], [1, L]]),
                        ap_of(ctile, (s_len - 1 - j) * L,
                              [[0, L - GROWS], [1, L]]),
                        op=alu.add,
                    )
                if fwd:
                    nc.vector.tensor_reduce(
                        ap_of(mstore, mslot * L, [[1, L]]), scf[:],
                        axis=mybir.AxisListType.X, op=alu.max,
                    )
                if j >= 0:
                    nc.vector.tensor_reduce(
                        ap_of(mstore, sslot * L, [[1, L]]), scb[:],
                        axis=mybir.AxisListType.X, op=alu.max,
                    )

                if fwd and j >= 0:
                    # [delta_ft | c_bt] = [mx_f | mx_b] + [e_ft | e_bt]
                    nc.vector.tensor_tensor(
                        ap_of(ctile, dout * L, [[(bt - dout) * L, 2], [1, L]]),
                        ap_of(mstore, mslot * L,
                              [[(sslot - mslot) * L, 2], [1, L]]),
                        ap_of(e_store, ft * L, [[(bt - ft) * L, 2], [1, L]]),
                        op=alu.add,
                    )
                elif fwd:
                    nc.vector.tensor_tensor(
                        ap_of(ctile, dout * L, [[1, L]]),
                        ap_of(mstore, mslot * L, [[1, L]]),
                        ap_of(e_store, ft * L, [[1, L]]),
                        op=alu.add,
                    )
                else:
                    nc.vector.tensor_tensor(
                        ap_of(ctile, bt * L, [[1, L]]),
                        ap_of(mstore, sslot * L, [[1, L]]),
                        ap_of(e_store, bt * L, [[1, L]]),
                        op=alu.add,
                    )

                if 1 <= i <= NF - 1:
                    scb_live[i] = emit_scb(i)

            # ---- Tail: gamma = mx_f + c (GpSimd), then first-wins onehot ----
            pat = singles.tile([BC, CH, L + 1], f32)
            nc.vector.memset(pat[:], 1.0)
            nc.vector.memset(pat[:, :, 0:1], 0.0)
            pad = singles.tile([BC, CH, L + 1], f32)
            nc.vector.memset(pad[:, :, 0:1], 0.0)

            for c0 in range(0, s_len, CH):
                if c0 + CH <= SPLIT_I + 1:
                    mx_ap = ap_of(mxf, c0 * L, [[L, CH], [1, L]])
                else:
                    mx_ap = ap_of(e_store, (s_len + 1 - c0) * L, [[-L, CH], [1, L]])
                g = gam_p.tile([BC, CH, L], f32, tag="g")
                nc.gpsimd.tensor_tensor(
                    g[:], c3[:, c0:c0 + CH, :], mx_ap, op=alu.add,
                )
                gm = tail1.tile([BC, CH], f32, tag="gm")
                nc.vector.reduce_max(gm[:], g[:], axis=mybir.AxisListType.X)
                gm_bc = (
                    gm[:]
                    .rearrange("p (t o) -> p t o", o=1)
                    .broadcast_to((BC, CH, L))
                )
                nc.vector.tensor_tensor(pad[:, :, 1:L + 1], g[:], gm_bc,
                                        op=alu.is_ge)
                scn = tail1.tile([BC, CH, L + 1], f32, tag="scn")
                nc.vector.tensor_tensor_scan(
                    out=scn[:].rearrange("p a b -> p (a b)"),
                    data0=pat[:].rearrange("p a b -> p (a b)"),
                    data1=pad[:].rearrange("p a b -> p (a b)"),
                    initial=0.0, op0=alu.mult, op1=alu.max,
                )
                fw = tail2.tile([BC, CH, L], f32, tag="fw")
                nc.vector.tensor_tensor(fw[:], pad[:, :, 1:L + 1],
                                        scn[:, :, 0:L], op=alu.is_gt)
                nc.sync.dma_start(Oh[:, c0:c0 + CH, :], fw[:])

    nc.compile()
    return nc


def _get(s_len):
    if s_len not in _BUILD_CACHE:
        _BUILD_CACHE[s_len] = _build(s_len)
    return _BUILD_CACHE[s_len]


LAST_RESULT = None


def kernel(X, W, T):
    global LAST_RESULT
    from concourse.bass_utils import run_bass_kernel_spmd

    X = np.ascontiguousarray(X, dtype=np.float32)
    W = np.ascontiguousarray(W, dtype=np.float32)
    T = np.ascontiguousarray(T, dtype=np.float32)
    s_len = X.shape[1]
    nc = _get(s_len)
    in_maps = [
        {"X": X[c * BC:(c + 1) * BC], "W": W, "T": T} for c in range(NCORES)
    ]
    res = run_bass_kernel_spmd(nc, in_maps, core_ids=list(range(NCORES)))
    LAST_RESULT = res
    return np.concatenate([r["OUT"] for r in res.results], axis=0)


# revision 12
# speedup vs baseline: 1.1402x; 1.1402x over previous
"""CRF Viterbi decode kernel for Trainium2 (8 NeuronCores, data-parallel over batch).

Per core (128 sequences, batch on partitions):
  Phase A: DMA X slabs (front/back interleaved) -> PE transpose -> PE matmul with W
           -> emissions e[b, s, l] (ACT copies PSUM->SBUF).
  Scan:    fwd delta-chain and bwd c-chain fused per iter, all on DVE
           (DVE+GpSimd share SBUF ports, so offloading elementwise work to
           GpSimd is zero-sum; a single in-order DVE queue has no sync
           stalls).  Iter i: sc = [T^T|T] + [delta_i|c_{511-i}] bcast,
           fused reduce -> [mx_f(i+1)|mx_b], fused out
           [delta_{i+1}|c_{510-i}] = mx + [e_{i+1}|e_{510-i}].
           mx_f(t) is stored per-t (mxf_store for t<288, dead e-slot 512-t
           after); c_t is stored per-t.  No ACT involvement in the scan.
  Tail:    gamma_t = mx_f(t) + c_t (GpSimd, chunked); onehot =
           first-wins(gamma == rowmax(gamma)) via segmented scan on DVE; DMA out.

gamma identity: delta_t = mx_f(t) + e_t, c_t = beta_t + e_t
  => delta_t + beta_t = mx_f(t) + c_t  (mx_f(0) := 0).
"""

import numpy as np

B, S, D, L = 1024, 512, 128, 26
NCORES = 8
BC = B // NCORES  # 128 sequences per core

_BUILD_CACHE = {}


def _build(s_len):
    import concourse.bass as bass
    import concourse.bacc as bacc
    import concourse.tile as tile
    import concourse.mybir as mybir

    alu = mybir.AluOpType
    f32 = mybir.dt.float32
    i32 = mybir.dt.int32

    nc = bacc.Bacc("TRN2", target_bir_lowering=False, debug=False)
    Xh = nc.dram_tensor("X", (BC, s_len, D), f32, kind="ExternalInput")
    Wh = nc.dram_tensor("W", (D, L), f32, kind="ExternalInput")
    Th = nc.dram_tensor("T", (L, L), f32, kind="ExternalInput")
    Oh = nc.dram_tensor("OUT", (BC, s_len, L), f32, kind="ExternalOutput")

    SCHUNK = 8 if s_len % 16 == 0 else s_len   # X staging granularity
    NCH = s_len // SCHUNK
    CH = 32 if s_len % 64 == 0 else s_len      # tail chunk (timesteps)
    NP = s_len - 1                             # scan iters
    DSLOT = s_len                              # ctile ping-pong base
    SPLIT_I = 287                              # first iter whose mx_f -> e_store
    MXN = SPLIT_I + 2                          # mxf slots (0=zeros, MXN-1=scratch)
    SCR2 = 250                                 # late-phase bwd-mx scratch e-slot

    def ap_of(t, offset_elems, dims):
        a = t[:]
        return bass.AP(tensor=a.tensor, offset=a.offset + offset_elems,
                       ap=[list(a.ap[0])] + dims)

    with tile.TileContext(nc) as tc:
        with (
            tc.tile_pool(name="singles", bufs=1) as singles,
            tc.tile_pool(name="xstage", bufs=2) as xstage_p,
            tc.tile_pool(name="xt", bufs=3) as xt_p,
            tc.tile_pool(name="ps_t", bufs=2, space="PSUM") as ps_t,
            tc.tile_pool(name="ps_e", bufs=2, space="PSUM") as ps_e,
            tc.tile_pool(name="sc", bufs=2) as sc_p,
            tc.tile_pool(name="gam", bufs=2) as gam_p,
            tc.tile_pool(name="tail1", bufs=1) as tail1,
            tc.tile_pool(name="tail2", bufs=2) as tail2,
        ):
            # ---- storage ----
            e_store = singles.tile([BC, s_len * L], f32)       # emissions
            ctile = singles.tile([BC, (s_len + 2) * L], f32)   # c_t + delta ping-pong
            mxf = singles.tile([BC, MXN * L], f32)             # mx_f(t), t <= SPLIT_I
            w_sb = singles.tile([D, L], f32)
            nc.sync.dma_start(w_sb[:], Wh[:])

            # t_cat[p, 0, j, i] = T[i, j]; t_cat[p, 1, i, j] = T[i, j]
            t_ap = Th[:]
            t_cat = singles.tile([BC, 2, L, L], f32)
            nc.sync.dma_start(
                t_cat[:, 1, :, :].rearrange("p a b -> p (a b)"),
                bass.AP(tensor=t_ap.tensor, offset=t_ap.offset,
                        ap=[[0, BC], [1, L * L]]),
            )
            nc.vector.tensor_copy(
                ap_of(t_cat, 0, [[L, L], [1, L]]),
                ap_of(t_cat, L * L, [[1, L], [L, L]]),
            )

            # identity matrix for PE transpose: ident[p, q] = (p == q)
            idx_i = singles.tile([BC, D], i32)
            nc.gpsimd.iota(idx_i[:], pattern=[[1, D]], base=0, channel_multiplier=0)
            pid_i = singles.tile([BC, 1], i32)
            nc.gpsimd.iota(pid_i[:], pattern=[[0, 1]], base=0, channel_multiplier=1)
            idx_f = singles.tile([BC, D], f32)
            nc.vector.tensor_copy(idx_f[:], idx_i[:])
            pid_f = singles.tile([BC, 1], f32)
            nc.vector.tensor_copy(pid_f[:], pid_i[:])
            ident = singles.tile([BC, D], f32)
            nc.vector.tensor_scalar(
                out=ident[:], in0=idx_f[:], scalar1=pid_f[:], scalar2=None,
                op0=alu.is_equal,
            )

            e3 = e_store.rearrange("p (s l) -> p s l", l=L)
            c3 = ctile.rearrange("p (s l) -> p s l", l=L)
            m3 = mxf.rearrange("p (s l) -> p s l", l=L)

            # ---- Phase A: emissions, front/back interleaved chunk order ----
            order = []
            for c in range(NCH // 2):
                order += [c, NCH - 1 - c]
            if NCH % 2:
                order.append(NCH // 2)
            for cidx in order:
                c0 = cidx * SCHUNK
                xs = xstage_p.tile([BC, SCHUNK, D], f32)
                nc.sync.dma_start(xs[:], Xh[:, c0:c0 + SCHUNK, :])
                for si in range(SCHUNK):
                    s = c0 + si
                    xt_psum = ps_t.tile([D, BC], f32)
                    nc.tensor.transpose(xt_psum[:], xs[:, si, :], ident[:])
                    xt_sb = xt_p.tile([D, BC], f32)
                    nc.scalar.copy(xt_sb[:], xt_psum[:])
                    e_psum = ps_e.tile([BC, L], f32)
                    nc.tensor.matmul(e_psum[:], lhsT=xt_sb[:], rhs=w_sb[:],
                                     start=True, stop=True)
                    nc.scalar.copy(e3[:, s, :], e_psum[:])

            # ---- init (DVE; the ACT queue is full of phase-A copies) ----
            nc.vector.memset(m3[:, 0, :], 0.0)                 # mx_f(0) := 0
            nc.vector.tensor_copy(c3[:, DSLOT, :], e3[:, 0, :])          # delta_0
            nc.vector.tensor_copy(c3[:, s_len - 1, :], e3[:, s_len - 1, :])  # c_{S-1}

            # ---- scan: iter i fuses fwd-step i and bwd-step i ----
            for i in range(NP):
                ft = i + 1          # fwd step produced (delta_ft, mx_f(ft))
                bt = s_len - 2 - i  # bwd step produced (c_bt)
                din = DSLOT + (i % 2)
                dout = DSLOT + ((i + 1) % 2)
                if i < SPLIT_I:
                    mstore, mslot, sslot = mxf, ft, MXN - 1
                else:
                    mstore, mslot, sslot = e_store, s_len - 1 - i, SCR2

                # sc[d,a,b]: d=0: T[b,a]+delta_i[b]; d=1: T[a,b]+c_{bt+1}[b]
                sc = sc_p.tile([BC, 2, L, L], f32, tag="sc")
                nc.vector.tensor_tensor(
                    sc[:], t_cat[:],
                    ap_of(ctile, din * L,
                          [[(bt + 1 - din) * L, 2], [0, L], [1, L]]),
                    op=alu.add,
                )
                nc.vector.tensor_reduce(
                    ap_of(mstore, mslot * L, [[(sslot - mslot) * L, 2], [1, L]]),
                    sc[:], axis=mybir.AxisListType.X, op=alu.max,
                )
                # [delta_ft | c_bt] = [mx_f | mx_b] + [e_ft | e_bt]
                nc.vector.tensor_tensor(
                    ap_of(ctile, dout * L, [[(bt - dout) * L, 2], [1, L]]),
                    ap_of(mstore, mslot * L, [[(sslot - mslot) * L, 2], [1, L]]),
                    ap_of(e_store, ft * L, [[(bt - ft) * L, 2], [1, L]]),
                    op=alu.add,
                )

            # ---- Tail: gamma = mx_f + c (GpSimd), then first-wins onehot ----
            pat = singles.tile([BC, CH, L + 1], f32)
            nc.vector.memset(pat[:], 1.0)
            nc.vector.memset(pat[:, :, 0:1], 0.0)
            pad = singles.tile([BC, CH, L + 1], f32)
            nc.vector.memset(pad[:, :, 0:1], 0.0)

            for c0 in range(0, s_len, CH):
                if c0 + CH <= SPLIT_I + 1:
                    mx_ap = ap_of(mxf, c0 * L, [[L, CH], [1, L]])
                else:
                    mx_ap = ap_of(e_store, (s_len - c0) * L, [[-L, CH], [1, L]])
                g = gam_p.tile([BC, CH, L], f32, tag="g")
                nc.gpsimd.tensor_tensor(
                    g[:], c3[:, c0:c0 + CH, :], mx_ap, op=alu.add,
                )
                gm = tail1.tile([BC, CH], f32, tag="gm")
                nc.vector.reduce_max(gm[:], g[:], axis=mybir.AxisListType.X)
                gm_bc = (
                    gm[:]
                    .rearrange("p (t o) -> p t o", o=1)
                    .broadcast_to((BC, CH, L))
                )
                nc.vector.tensor_tensor(pad[:, :, 1:L + 1], g[:], gm_bc,
                                        op=alu.is_ge)
                scn = tail1.tile([BC, CH, L + 1], f32, tag="scn")
                nc.vector.tensor_tensor_scan(
                    out=scn[:].rearrange("p a b -> p (a b)"),
                    data0=pat[:].rearrange("p a b -> p (a b)"),
                    data1=pad[:].rearrange("p a b -> p (a b)"),
                    initial=0.0, op0=alu.mult, op1=alu.max,
                )
                fw = tail2.tile([BC, CH, L], f32, tag="fw")
                nc.vector.tensor_tensor(fw[:], pad[:, :, 1:L + 1],
                                        scn[:, :, 0:L], op=alu.is_gt)
                nc.sync.dma_start(Oh[:, c0:c0 + CH, :], fw[:])

    nc.compile()
    return nc


def _get(s_len):
    if s_len not in _BUILD_CACHE:
        _BUILD_CACHE[s_len] = _build(s_len)
    return _BUILD_CACHE[s_len]


LAST_RESULT = None


def kernel(X, W, T):
    global LAST_RESULT
    from concourse.bass_utils import run_bass_kernel_spmd

    X = np.ascontiguousarray(X, dtype=np.float32)
    W = np.ascontiguousarray(W, dtype=np.float32)
    T = np.ascontiguousarray(T, dtype=np.float32)
    s_len = X.shape[1]
    nc = _get(s_len)
    in_maps = [
        {"X": X[c * BC:(c + 1) * BC], "W": W, "T": T} for c in range(NCORES)
    ]
    res = run_bass_kernel_spmd(nc, in_maps, core_ids=list(range(NCORES)))
    LAST_RESULT = res
    return np.concatenate([r["OUT"] for r in res.results], axis=0)


# revision 14
# speedup vs baseline: 1.1404x; 1.0001x over previous
"""CRF Viterbi decode kernel for Trainium2 (8 NeuronCores, data-parallel over batch).

Per core (128 sequences, batch on partitions):
  Phase A: DMA X slabs (front/back interleaved) -> PE transpose -> PE matmul with W
           -> emissions e[b, s, l] (ACT copies PSUM->SBUF).
  Scan:    fwd delta-chain and bwd c-chain fused per iter, all on DVE
           (DVE+GpSimd share SBUF ports, so offloading elementwise work to
           GpSimd is zero-sum; a single in-order DVE queue has no sync
           stalls).  Iter i: sc = [T^T|T] + [delta_i|c_{511-i}] bcast,
           fused reduce -> [mx_f(i+1)|mx_b], fused out
           [delta_{i+1}|c_{510-i}] = mx + [e_{i+1}|e_{510-i}].
           mx_f(t) is stored per-t (mxf_store for t<288, dead e-slot 512-t
           after); c_t is stored per-t.  No ACT involvement in the scan.
  Tail:    gamma_t = mx_f(t) + c_t (GpSimd, chunked); onehot =
           first-wins(gamma == rowmax(gamma)) via segmented scan on DVE; DMA out.

gamma identity: delta_t = mx_f(t) + e_t, c_t = beta_t + e_t
  => delta_t + beta_t = mx_f(t) + c_t  (mx_f(0) := 0).
"""

import numpy as np

B, S, D, L = 1024, 512, 128, 26
NCORES = 8
BC = B // NCORES  # 128 sequences per core

_BUILD_CACHE = {}


def _build(s_len):
    import concourse.bass as bass
    import concourse.bacc as bacc
    import concourse.tile as tile
    import concourse.mybir as mybir

    alu = mybir.AluOpType
    f32 = mybir.dt.float32
    i32 = mybir.dt.int32

    nc = bacc.Bacc("TRN2", target_bir_lowering=False, debug=False)
    Xh = nc.dram_tensor("X", (BC, s_len, D), f32, kind="ExternalInput")
    Wh = nc.dram_tensor("W", (D, L), f32, kind="ExternalInput")
    Th = nc.dram_tensor("T", (L, L), f32, kind="ExternalInput")
    Oh = nc.dram_tensor("OUT", (BC, s_len, L), f32, kind="ExternalOutput")

    SCHUNK = 8 if s_len % 16 == 0 else s_len   # X staging granularity
    NCH = s_len // SCHUNK
    CH = 64 if s_len % 64 == 0 else s_len      # tail chunk (timesteps)
    NP = s_len - 1                             # scan iters
    DSLOT = s_len                              # ctile ping-pong base
    SPLIT_I = 319                              # first iter whose mx_f -> e_store
    MXN = SPLIT_I + 2                          # mxf slots (0=zeros, MXN-1=scratch)
    SCR2 = 250                                 # late-phase bwd-mx scratch e-slot

    def ap_of(t, offset_elems, dims):
        a = t[:]
        return bass.AP(tensor=a.tensor, offset=a.offset + offset_elems,
                       ap=[list(a.ap[0])] + dims)

    with tile.TileContext(nc) as tc:
        with (
            tc.tile_pool(name="singles", bufs=1) as singles,
            tc.tile_pool(name="xstage", bufs=2) as xstage_p,
            tc.tile_pool(name="xt", bufs=3) as xt_p,
            tc.tile_pool(name="ps_t", bufs=2, space="PSUM") as ps_t,
            tc.tile_pool(name="ps_e", bufs=2, space="PSUM") as ps_e,
            tc.tile_pool(name="sc", bufs=2) as sc_p,
            tc.tile_pool(name="gam", bufs=2) as gam_p,
            tc.tile_pool(name="tail1", bufs=1) as tail1,
            tc.tile_pool(name="tail2", bufs=1) as tail2,
        ):
            # ---- storage ----
            e_store = singles.tile([BC, s_len * L], f32)       # emissions
            ctile = singles.tile([BC, (s_len + 2) * L], f32)   # c_t + delta ping-pong
            mxf = singles.tile([BC, MXN * L], f32)             # mx_f(t), t <= SPLIT_I
            w_sb = singles.tile([D, L], f32)
            nc.sync.dma_start(w_sb[:], Wh[:])

            # t_cat[p, 0, j, i] = T[i, j]; t_cat[p, 1, i, j] = T[i, j]
            t_ap = Th[:]
            t_cat = singles.tile([BC, 2, L, L], f32)
            nc.sync.dma_start(
                t_cat[:, 1, :, :].rearrange("p a b -> p (a b)"),
                bass.AP(tensor=t_ap.tensor, offset=t_ap.offset,
                        ap=[[0, BC], [1, L * L]]),
            )
            nc.vector.tensor_copy(
                ap_of(t_cat, 0, [[L, L], [1, L]]),
                ap_of(t_cat, L * L, [[1, L], [L, L]]),
            )

            # identity matrix for PE transpose: ident[p, q] = (p == q)
            idx_i = singles.tile([BC, D], i32)
            nc.gpsimd.iota(idx_i[:], pattern=[[1, D]], base=0, channel_multiplier=0)
            pid_i = singles.tile([BC, 1], i32)
            nc.gpsimd.iota(pid_i[:], pattern=[[0, 1]], base=0, channel_multiplier=1)
            idx_f = singles.tile([BC, D], f32)
            nc.vector.tensor_copy(idx_f[:], idx_i[:])
            pid_f = singles.tile([BC, 1], f32)
            nc.vector.tensor_copy(pid_f[:], pid_i[:])
            ident = singles.tile([BC, D], f32)
            nc.vector.tensor_scalar(
                out=ident[:], in0=idx_f[:], scalar1=pid_f[:], scalar2=None,
                op0=alu.is_equal,
            )

            e3 = e_store.rearrange("p (s l) -> p s l", l=L)
            c3 = ctile.rearrange("p (s l) -> p s l", l=L)
            m3 = mxf.rearrange("p (s l) -> p s l", l=L)

            # ---- Phase A: emissions, front/back interleaved chunk order ----
            order = []
            for c in range(NCH // 2):
                order += [c, NCH - 1 - c]
            if NCH % 2:
                order.append(NCH // 2)
            for cidx in order:
                c0 = cidx * SCHUNK
                xs = xstage_p.tile([BC, SCHUNK, D], f32)
                nc.sync.dma_start(xs[:], Xh[:, c0:c0 + SCHUNK, :])
                for si in range(SCHUNK):
                    s = c0 + si
                    xt_psum = ps_t.tile([D, BC], f32)
                    nc.tensor.transpose(xt_psum[:], xs[:, si, :], ident[:])
                    xt_sb = xt_p.tile([D, BC], f32)
                    nc.scalar.copy(xt_sb[:], xt_psum[:])
                    e_psum = ps_e.tile([BC, L], f32)
                    nc.tensor.matmul(e_psum[:], lhsT=xt_sb[:], rhs=w_sb[:],
                                     start=True, stop=True)
                    nc.scalar.copy(e3[:, s, :], e_psum[:])

            # ---- init (DVE; the ACT queue is full of phase-A copies) ----
            nc.vector.memset(m3[:, 0, :], 0.0)                 # mx_f(0) := 0
            nc.vector.tensor_copy(c3[:, DSLOT, :], e3[:, 0, :])          # delta_0
            nc.vector.tensor_copy(c3[:, s_len - 1, :], e3[:, s_len - 1, :])  # c_{S-1}

            # ---- scan: iter i fuses fwd-step i and bwd-step i ----
            for i in range(NP):
                ft = i + 1          # fwd step produced (delta_ft, mx_f(ft))
                bt = s_len - 2 - i  # bwd step produced (c_bt)
                din = DSLOT + (i % 2)
                dout = DSLOT + ((i + 1) % 2)
                if i < SPLIT_I:
                    mstore, mslot, sslot = mxf, ft, MXN - 1
                else:
                    mstore, mslot, sslot = e_store, s_len - 1 - i, SCR2

                # sc[d,a,b]: d=0: T[b,a]+delta_i[b]; d=1: T[a,b]+c_{bt+1}[b]
                sc = sc_p.tile([BC, 2, L, L], f32, tag="sc")
                nc.vector.tensor_tensor(
                    sc[:], t_cat[:],
                    ap_of(ctile, din * L,
                          [[(bt + 1 - din) * L, 2], [0, L], [1, L]]),
                    op=alu.add,
                )
                nc.vector.tensor_reduce(
                    ap_of(mstore, mslot * L, [[(sslot - mslot) * L, 2], [1, L]]),
                    sc[:], axis=mybir.AxisListType.X, op=alu.max,
                )
                # [delta_ft | c_bt] = [mx_f | mx_b] + [e_ft | e_bt]
                nc.vector.tensor_tensor(
                    ap_of(ctile, dout * L, [[(bt - dout) * L, 2], [1, L]]),
                    ap_of(mstore, mslot * L, [[(sslot - mslot) * L, 2], [1, L]]),
                    ap_of(e_store, ft * L, [[(bt - ft) * L, 2], [1, L]]),
                    op=alu.add,
                )

            # ---- Tail: gamma = mx_f + c (GpSimd), then first-wins onehot ----
            pat = singles.tile([BC, CH, L + 1], f32)
            nc.vector.memset(pat[:], 1.0)
            nc.vector.memset(pat[:, :, 0:1], 0.0)
            pad = singles.tile([BC, CH, L + 1], f32)
            nc.vector.memset(pad[:, :, 0:1], 0.0)

            for c0 in range(0, s_len, CH):
                if c0 + CH <= SPLIT_I + 1:
                    mx_ap = ap_of(mxf, c0 * L, [[L, CH], [1, L]])
                else:
                    mx_ap = ap_of(e_store, (s_len - c0) * L, [[-L, CH], [1, L]])
                g = gam_p.tile([BC, CH, L], f32, tag="g")
                nc.gpsimd.tensor_tensor(
                    g[:], c3[:, c0:c0 + CH, :], mx_ap, op=alu.add,
                )
                gm = tail1.tile([BC, CH], f32, tag="gm")
                nc.vector.reduce_max(gm[:], g[:], axis=mybir.AxisListType.X)
                gm_bc = (
                    gm[:]
                    .rearrange("p (t o) -> p t o", o=1)
                    .broadcast_to((BC, CH, L))
                )
                nc.vector.tensor_tensor(pad[:, :, 1:L + 1], g[:], gm_bc,
                                        op=alu.is_ge)
                scn = tail1.tile([BC, CH, L + 1], f32, tag="scn")
                nc.vector.tensor_tensor_scan(
                    out=scn[:].rearrange("p a b -> p (a b)"),
                    data0=pat[:].rearrange("p a b -> p (a b)"),
                    data1=pad[:].rearrange("p a b -> p (a b)"),
                    initial=0.0, op0=alu.mult, op1=alu.max,
                )
                fw = tail2.tile([BC, CH, L], f32, tag="fw")
                nc.vector.tensor_tensor(fw[:], pad[:, :, 1:L + 1],
                                        scn[:, :, 0:L], op=alu.is_gt)
                nc.sync.dma_start(Oh[:, c0:c0 + CH, :], fw[:])

    nc.compile()
    return nc


def _get(s_len):
    if s_len not in _BUILD_CACHE:
        _BUILD_CACHE[s_len] = _build(s_len)
    return _BUILD_CACHE[s_len]


LAST_RESULT = None


def kernel(X, W, T):
    global LAST_RESULT
    from concourse.bass_utils import run_bass_kernel_spmd

    X = np.ascontiguousarray(X, dtype=np.float32)
    W = np.ascontiguousarray(W, dtype=np.float32)
    T = np.ascontiguousarray(T, dtype=np.float32)
    s_len = X.shape[1]
    nc = _get(s_len)
    in_maps = [
        {"X": X[c * BC:(c + 1) * BC], "W": W, "T": T} for c in range(NCORES)
    ]
    res = run_bass_kernel_spmd(nc, in_maps, core_ids=list(range(NCORES)))
    LAST_RESULT = res
    return np.concatenate([r["OUT"] for r in res.results], axis=0)
